# revision 5
# baseline (speedup 1.0000x reference)
"""2D DCT [8,32,256,256] on 8 TRN2 NeuronCores — raw Bass (no Tile).

Math: with A[m,k] = cos(pi*k*(m+0.5)/L)/L the 2D DCT per [256,256] slice is
    out = A^T @ X @ A
Stage 1: V = X^T A via 4 matmuls N=256 per slice (lhsT = X h-chunks,
rhs = A), one PSUM bank per slice. The host stages the second half of the
w columns REVERSED, so the bank holds
    vp[v, 0:256]   = V[v, j]        (v = 0..127)
    vp[v, 256:512] = V[255-v, j]
Stage 2 uses the DCT-II even/odd symmetry A[255-v, w'] = (-1)^w' A[v, w']:
    out[j, 2t']   = sum_v (V[v,j]+V[255-v,j]) E2[v,t']
    out[j, 2t'+1] = sum_v (V[v,j]-V[255-v,j]) O2[v,t']
so stage 2 is 2 matmuls of N=512 PER SLICE PAIR (lhsT = E2/O2 stationary,
contraction 128) — 1536 streamed PE columns per slice vs the dense
baseline's 2048.

The butterfly folds cannot read two PSUM operands (walrus NCC_IBVF027),
so the per-pair pipeline is:
    PE S1(2p),S1(2p+1) -> vp banks (f32)
    ACT cast-pair      vp -> vs_f bf16 (FD=1024, ~1.1us)
    DVE fold-batch     vs_f -> vs_sd bf16 (2 pairs/op, 2x mode, FD=1024)
    PE S2(p)           vs_sd -> op banks (2 matmuls N=512)
    DVE/ACT out-evict  op -> os bf16 (FD=1024; ACT p%3==0-ish, DVE rest)
    sync-ring DMA      os -> DRAM        (ACT DMAs the tail pair inline)

Implied-wait discipline (waits cost NX time and break LDWEIGHTS
pull-ahead): PE block p = [wait act>=cast(p-2)] S1(2p) S1(2p+1)
[wait dve>=fold(p-LAG)] S2(p-LAG) with LAG=3; out-evicts are emitted
BEFORE the cast/fold of the 2-later pair in their engine stream so the two
PE waits transitively imply vp-bank and op-bank recycling. vs_f ring 8 /
vs_sd ring 6 make the fold-side recycles implied as well. Never let two
agents touch one PSUM bank concurrently (hard device crash) — the
act>=cast(p-2) wait before each S1 pair is the load-bearing guard.

Measured primitive paces (this container, warm K=8/8 @2.4GHz): N=256
matmul 109ns, N=512 matmul 216ns (lhsT reuse, LDW hidden), ACT copy
FD/1.2+143ns, DVE cast FD/0.96+65ns, DVE bf16 TT 2x FD/1.92+69ns.
"""

import numpy as np

import concourse.bacc as bacc
import concourse.bass as bass
import concourse.mybir as mybir
from concourse.bass_utils import run_bass_kernel_spmd

N_CORES = 8
C = 32                    # slices per core
P = 16                    # slice pairs per core
L = 256
BF16 = mybir.dt.bfloat16
F32 = mybir.dt.float32
NP_BF16 = mybir.dt.np(mybir.dt.bfloat16)

# staged input units: 0 = A, 1 = [E2|O2], 2+s = slice s
IN_CHUNKS = [3, 1, 1, 1, 2, 2, 3, 5, 8, 8]        # 34 units
OUT_CHUNKS = [3, 3, 3, 3, 2, 1]                   # pairs 0..14 on sync ring
TAIL_PAIR = 15                                    # pair 15 DMA'd from ACT
N_WARM = 26
VPR = 4                   # vp ring (banks) — slice s -> bank s%4
OPR = 4                   # op ring — pair p -> banks 2*(p%2), 2*(p%2)+1
VFR = 8                   # vs_f ring slots — slice s -> slot s%8
SDR = 6                   # vs_sd ring — pair p -> slot p%6
LAG = 3                   # S2(p-LAG) in PE pair block p
# out-evict engine: ACT ~5 of 16 (it also does all casts), DVE the rest
OUT_ENG = ["act" if p % 3 == 0 and p > 0 else "dve" for p in range(P)]
OUT_ENG[TAIL_PAIR] = "act"                        # ACT issues tail DMA


def _dct_matrix() -> np.ndarray:
    m = np.arange(L, dtype=np.float64)
    k = np.arange(L, dtype=np.float64)
    a = np.cos(np.pi * np.outer(m + 0.5, k) / L) / L
    return a.astype(np.float32)


def _chunk_of_slice(s):
    u = s + 2
    c0 = 0
    for ci, n in enumerate(IN_CHUNKS):
        if u < c0 + n:
            return ci
        c0 += n
    raise AssertionError


def _schedules():
    """Per-engine op orders + completion counts (sem value when done)."""
    pe = []
    for p in range(P):
        pe.append(("S1", 2 * p))
        pe.append(("S1", 2 * p + 1))
        if p >= LAG:
            pe.append(("S2", p - LAG))
    for p in range(P - LAG, P):
        pe.append(("S2", p))
    pe_count = {o: i + 1 for i, o in enumerate(pe)}

    # casts/folds lead their stream (critical path); out-evicts trail
    act = []
    for p in range(P):
        act.append(("cast", p))
        if p >= 2 and OUT_ENG[p - 2] == "act":
            act.append(("out", p - 2))
    for q in (P - 2, P - 1):
        if OUT_ENG[q] == "act":
            act.append(("out", q))
    act_count = {o: i + 1 for i, o in enumerate(act)}

    dve = []
    for k in range(P // 2):
        dve.append(("foldb", k))
        for q in (2 * k - 2, 2 * k - 1):
            if 0 <= q and OUT_ENG[q] == "dve":
                dve.append(("out", q))
    for q in (P - 2, P - 1):
        if OUT_ENG[q] == "dve":
            dve.append(("out", q))
    dve_count = {o: i + 1 for i, o in enumerate(dve)}
    return pe, pe_count, act, act_count, dve, dve_count


def _build(sim: bool = False) -> bass.Bass:
    nc = bacc.Bacc()
    x = nc.declare_dram_parameter("x", [128, 2 + C, 512], BF16, isOutput=False)
    out = nc.declare_dram_parameter("out", [128, P, 2, 512], BF16, isOutput=True)

    pe, pe_count, act, act_count, dve, dve_count = _schedules()

    from contextlib import ExitStack

    ctx = ExitStack()
    with ctx:
        warm_sb = ctx.enter_context(nc.sbuf_tensor([128, 128], BF16))
        xs = ctx.enter_context(nc.sbuf_tensor([128, 2 + C, 512], BF16))
        vs_f = ctx.enter_context(nc.sbuf_tensor([128, VFR, 512], BF16))
        vs_sd = ctx.enter_context(nc.sbuf_tensor([128, SDR, 2, 2, 256], BF16))
        os_ = ctx.enter_context(nc.sbuf_tensor([128, P, 2, 512], BF16))
        vp = ctx.enter_context(nc.psum_tensor([128, VPR, 512], F32))
        op = ctx.enter_context(nc.psum_tensor([128, OPR, 512], F32))

        in_sems = [
            ctx.enter_context(nc.semaphore(f"in_sem{i}"))
            for i in range(len(IN_CHUNKS))
        ]
        pe_sem = ctx.enter_context(nc.semaphore("pe_sem"))
        dve_sem = ctx.enter_context(nc.semaphore("dve_sem"))
        act_sem = ctx.enter_context(nc.semaphore("act_sem"))
        out_sem = ctx.enter_context(nc.semaphore("out_sem"))
        warm_sem = ctx.enter_context(nc.semaphore("warm_sem"))
        sem_of = {"dve": dve_sem, "act": act_sem}
        count_of = {"dve": dve_count, "act": act_count}

        block = ctx.enter_context(nc.Block())

        @block.sync
        def _(eng):
            u0 = 0
            for ci, n in enumerate(IN_CHUNKS):
                eng.dma_start(
                    xs[:, u0 : u0 + n, :], x[:, u0 : u0 + n, :]
                ).then_inc(in_sems[ci], 16)
                u0 += n
            c0 = 0
            for n in OUT_CHUNKS:
                for eng_name in ("dve", "act"):
                    need = max(
                        (
                            count_of[eng_name][("out", q)]
                            for q in range(c0, c0 + n)
                            if OUT_ENG[q] == eng_name
                        ),
                        default=0,
                    )
                    if need:
                        eng.wait_ge(sem_of[eng_name], need)
                eng.dma_start(
                    out[:, c0 : c0 + n, :, :], os_[:, c0 : c0 + n, :, :]
                ).then_inc(out_sem, 16)
                c0 += n
            eng.wait_ge(out_sem, 16 * (len(OUT_CHUNKS) + 1))

        @block.tensor
        def _(eng):
            if sim:
                eng.wait_ge(warm_sem, 1)
            for _ in range(N_WARM):
                nc.tensor.matmul(
                    vp[:, 0, 0:128], warm_sb[:], warm_sb[:],
                    start=True, stop=True,
                )
            eng.wait_ge(in_sems[0], 16)
            seen_chunks = {0}
            for kind, i in pe:
                if kind == "S1":
                    s = i
                    ci = _chunk_of_slice(s)
                    if ci not in seen_chunks:
                        seen_chunks.add(ci)
                        eng.wait_ge(in_sems[ci], 16)
                    if s % 2 == 0 and s >= 4:
                        # vp banks for pair s//2 freed by cast(s//2 - 2);
                        # this wait is the PSUM two-agent guard
                        eng.wait_ge(act_sem, act_count[("cast", s // 2 - 2)])
                    r = s % VPR
                    for mi in range(2):
                        for ki in range(2):
                            mm = nc.tensor.matmul(
                                vp[:, r, mi * 256 : (mi + 1) * 256],
                                xs[:, 2 + s, ki * 256 + mi * 128 : ki * 256 + (mi + 1) * 128],
                                xs[:, 0, ki * 256 : (ki + 1) * 256],
                                start=(ki == 0),
                                stop=(ki == 1),
                            )
                    mm.then_inc(pe_sem, 1)
                else:
                    q = i
                    # fold-batch(q//2) done implies cast(q) and upstream;
                    # op-bank recycle needs out(q-2) explicitly
                    eng.wait_ge(dve_sem, dve_count[("foldb", q // 2)])
                    if q >= 2:
                        e = OUT_ENG[q - 2]
                        eng.wait_ge(sem_of[e], count_of[e][("out", q - 2)])
                    b0 = 2 * (q % 2)
                    nc.tensor.matmul(
                        op[:, b0, :],
                        xs[:, 1, 0:128],
                        vs_sd[:, q % SDR, 0, :, :],
                        start=True, stop=True,
                    )
                    mm = nc.tensor.matmul(
                        op[:, b0 + 1, :],
                        xs[:, 1, 128:256],
                        vs_sd[:, q % SDR, 1, :, :],
                        start=True, stop=True,
                    )
                    mm.then_inc(pe_sem, 1)

        @block.scalar
        def _(eng):
            for kind, p in act:
                if kind == "cast":
                    eng.wait_ge(pe_sem, pe_count[("S1", 2 * p + 1)])
                    cp = nc.scalar.copy(
                        vs_f[:, (2 * p) % VFR : (2 * p) % VFR + 2, :],
                        vp[:, (2 * p) % VPR : (2 * p) % VPR + 2, :],
                    )
                else:
                    eng.wait_ge(pe_sem, pe_count[("S2", p)])
                    cp = nc.scalar.copy(
                        os_[:, p, :, :],
                        op[:, 2 * (p % 2) : 2 * (p % 2) + 2, :],
                    )
                cp.then_inc(act_sem, 1)
            # tail out-DMA for the last pair (its eviction just ran here)
            eng.dma_start(
                out[:, TAIL_PAIR, :, :], os_[:, TAIL_PAIR, :, :]
            ).then_inc(out_sem, 16)

        @block.vector
        def _(eng):
            add = mybir.AluOpType.add
            sub = mybir.AluOpType.subtract
            if sim:
                nc.vector.memset(warm_sb[:], 0.0).then_inc(warm_sem, 1)
            for kind, i in dve:
                if kind == "foldb":
                    k = i
                    # covers pairs 2k, 2k+1 (slices 4k..4k+3)
                    eng.wait_ge(act_sem, act_count[("cast", 2 * k + 1)])
                    f0 = (4 * k) % VFR
                    q0 = (2 * k) % SDR
                    nc.vector.tensor_tensor(
                        vs_sd[:, q0 : q0 + 2, 0, :, :],
                        vs_f[:, f0 : f0 + 4, 0:256],
                        vs_f[:, f0 : f0 + 4, 256:512],
                        add,
                    )
                    tt = nc.vector.tensor_tensor(
                        vs_sd[:, q0 : q0 + 2, 1, :, :],
                        vs_f[:, f0 : f0 + 4, 0:256],
                        vs_f[:, f0 : f0 + 4, 256:512],
                        sub,
                    )
                    tt.then_inc(dve_sem, 1)
                else:
                    q = i
                    eng.wait_ge(pe_sem, pe_count[("S2", q)])
                    nc.vector.tensor_copy(
                        os_[:, q, :, :],
                        op[:, 2 * (q % 2) : 2 * (q % 2) + 2, :],
                    ).then_inc(dve_sem, 1)

    nc.compile()
    return nc


_NC_CACHE: bass.Bass | None = None


def _get_nc() -> bass.Bass:
    global _NC_CACHE
    if _NC_CACHE is None:
        _NC_CACHE = _build()
    return _NC_CACHE


def _make_in_maps(ip: np.ndarray) -> list[dict[str, np.ndarray]]:
    a = _dct_matrix()                                   # [256, 256] f32
    a_bf = a.astype(NP_BF16)
    unit_a = (
        a_bf.reshape(2, 128, 256).transpose(1, 0, 2).reshape(128, 512)
    )                                                   # [p, ki*256+j]
    unit_eo = np.zeros((128, 512), dtype=NP_BF16)
    unit_eo[:, 0:128] = a_bf[0:128, 0::2]               # E2[v, t']
    unit_eo[:, 128:256] = a_bf[0:128, 1::2]             # O2[v, t']
    in_maps = []
    for b in range(N_CORES):
        xb = ip[b].astype(NP_BF16)                      # [C, 256, 256]
        # w-permutation: cols 128.. hold w = 255..128
        xp = np.concatenate([xb[:, :, :128], xb[:, :, 128:][:, :, ::-1]], axis=2)
        # [s, ki, p, mi, c] -> [p, s, ki*256+mi*128+c]
        st = xp.reshape(C, 2, 128, 2, 128).transpose(2, 0, 1, 3, 4).reshape(128, C, 512)
        full = np.concatenate(
            [unit_a[:, None, :], unit_eo[:, None, :], st], axis=1
        )                                               # [128, 34, 512]
        in_maps.append({"x": np.ascontiguousarray(full)})
    return in_maps


def _unpack_out(results: list[dict[str, np.ndarray]]) -> np.ndarray:
    outs = []
    for b in range(N_CORES):
        o = np.asarray(results[b]["out"]).astype(np.float32)  # [128,16,2,512]
        o = o.reshape(128, P, 2, 2, 256)                # [t', pair, eo, sb, j]
        o = o.transpose(1, 3, 4, 0, 2).reshape(C, 256, 256)  # [s, j, w'=2t'+eo]
        outs.append(o)
    return np.stack(outs, axis=0)


def run(ip: np.ndarray, trace: bool = False):
    """Run the device kernel; returns (output, BassKernelResults)."""
    ip = np.asarray(ip)
    assert ip.shape == (N_CORES, C, 256, 256), ip.shape
    res = run_bass_kernel_spmd(
        _get_nc(), _make_in_maps(ip), core_ids=list(range(N_CORES)), trace=trace
    )
    return _unpack_out(res.results), res


def kernel(ip: np.ndarray) -> np.ndarray:
    out, _ = run(ip)
    return out


# revision 11
# speedup vs baseline: 1.0750x; 1.0750x over previous
"""2D DCT [8,32,256,256] on 8 TRN2 NeuronCores — raw Bass (no Tile).

Math: with A[m,k] = cos(pi*k*(m+0.5)/L)/L the 2D DCT per [256,256] slice is
    out = A^T @ X @ A
Stage 1: V = X^T A via 4 matmuls N=256 per slice (lhsT = X h-chunks,
rhs = A), one PSUM bank per slice. The host stages the second half of the
w columns REVERSED, so the bank holds
    vp[v, 0:256]   = V[v, j]        (v = 0..127)
    vp[v, 256:512] = V[255-v, j]
Stage 2 uses the DCT-II even/odd symmetry A[255-v, w'] = (-1)^w' A[v, w']:
    out[j, 2t']   = sum_v (V[v,j]+V[255-v,j]) E2[v,t']
    out[j, 2t'+1] = sum_v (V[v,j]-V[255-v,j]) O2[v,t']
so stage 2 is 2 matmuls of N=512 PER SLICE PAIR (lhsT = E2/O2 stationary,
contraction 128) — 1536 streamed PE columns per slice vs the dense
baseline's 2048.

The butterfly folds cannot read two PSUM operands (walrus NCC_IBVF027),
so the per-pair pipeline is:
    PE S1(2p),S1(2p+1) -> vp banks (f32)
    ACT cast-pair      vp -> vs_f bf16 (FD=1024, ~1.1us)
    DVE fold-batch     vs_f -> vs_sd bf16 (2 pairs/op, 2x mode, FD=1024)
    PE S2(p)           vs_sd -> op banks (2 matmuls N=512)
    DVE/ACT out-evict  op -> os bf16 (FD=1024; ACT p%3==0-ish, DVE rest)
    sync-ring DMA      os -> DRAM        (ACT DMAs the tail pair inline)

Implied-wait discipline (waits cost NX time and break LDWEIGHTS
pull-ahead): PE block p = [wait act>=cast(p-2)] S1(2p) S1(2p+1)
[wait dve>=fold(p-LAG)] S2(p-LAG) with LAG=3; out-evicts are emitted
BEFORE the cast/fold of the 2-later pair in their engine stream so the two
PE waits transitively imply vp-bank and op-bank recycling. vs_f ring 8 /
vs_sd ring 6 make the fold-side recycles implied as well. Never let two
agents touch one PSUM bank concurrently (hard device crash) — the
act>=cast(p-2) wait before each S1 pair is the load-bearing guard.

Measured primitive paces (this container, warm K=8/8 @2.4GHz): N=256
matmul 109ns, N=512 matmul 216ns (lhsT reuse, LDW hidden), ACT copy
FD/1.2+143ns, DVE cast FD/0.96+65ns, DVE bf16 TT 2x FD/1.92+69ns.
"""

import numpy as np

import concourse.bacc as bacc
import concourse.bass as bass
import concourse.mybir as mybir
from concourse.bass_utils import run_bass_kernel_spmd

N_CORES = 8
C = 32                    # slices per core
P = 16                    # slice pairs per core
L = 256
BF16 = mybir.dt.bfloat16
F32 = mybir.dt.float32
NP_BF16 = mybir.dt.np(mybir.dt.bfloat16)

# staged input units: 0 = A, 1 = [E2|O2], 2+s = slice s
IN_CHUNKS = [3, 1, 1, 1, 2, 2, 3, 5, 8, 8]        # 34 units
OUT_CHUNKS = [3, 3, 3, 3, 2, 1]                   # pairs 0..14 on sync ring
TAIL_PAIR = 15                                    # pair 15 DMA'd from ACT
N_WARM = 40
VPR = 4                   # vp ring (banks) — slice s -> bank s%4
OPR = 4                   # op ring — pair p -> banks 2*(p%2), 2*(p%2)+1
VFR = 8                   # vs_f ring slots — slice s -> slot s%8
SDR = 6                   # vs_sd ring — pair p -> slot p%6
LAG = 3                   # S2(p-LAG) in PE pair block p
# out-evict engine: ACT ~5 of 16 (it also does all casts), DVE the rest
OUT_ENG = ["act" if p % 3 == 0 and p > 0 else "dve" for p in range(P)]
OUT_ENG[TAIL_PAIR] = "act"                        # ACT issues tail DMA


def _dct_matrix() -> np.ndarray:
    m = np.arange(L, dtype=np.float64)
    k = np.arange(L, dtype=np.float64)
    a = np.cos(np.pi * np.outer(m + 0.5, k) / L) / L
    return a.astype(np.float32)


def _chunk_of_slice(s):
    u = s + 2
    c0 = 0
    for ci, n in enumerate(IN_CHUNKS):
        if u < c0 + n:
            return ci
        c0 += n
    raise AssertionError


def _schedules():
    """Per-engine op orders + completion counts (sem value when done)."""
    pe = []
    for p in range(P):
        pe.append(("S1", 2 * p))
        pe.append(("S1", 2 * p + 1))
        if p >= LAG:
            pe.append(("S2", p - LAG))
    for p in range(P - LAG, P):
        pe.append(("S2", p))
    pe_count = {o: i + 1 for i, o in enumerate(pe)}

    # per-SLICE casts lead the ACT stream (lowest S1->cast latency);
    # out-evicts trail behind the cast of the 2-later pair
    act = []
    for s in range(2 * P):
        act.append(("cast", s))
        if s % 2 == 1:
            q = s // 2 - 2
            if q >= 0 and OUT_ENG[q] == "act":
                act.append(("out", q))
    for q in (P - 2, P - 1):
        if OUT_ENG[q] == "act":
            act.append(("out", q))
    act_count = {o: i + 1 for i, o in enumerate(act)}

    # per-PAIR folds (fold p fires right after cast(2p+1))
    dve = []
    for p in range(P):
        dve.append(("fold", p))
        q = p - 2
        if q >= 0 and OUT_ENG[q] == "dve":
            dve.append(("out", q))
    for q in (P - 2, P - 1):
        if OUT_ENG[q] == "dve":
            dve.append(("out", q))
    dve_count = {o: i + 1 for i, o in enumerate(dve)}
    return pe, pe_count, act, act_count, dve, dve_count


def _build(sim: bool = False) -> bass.Bass:
    nc = bacc.Bacc()
    x = nc.declare_dram_parameter("x", [128, 2 + C, 512], BF16, isOutput=False)
    out = nc.declare_dram_parameter("out", [128, P, 2, 512], BF16, isOutput=True)

    pe, pe_count, act, act_count, dve, dve_count = _schedules()

    from contextlib import ExitStack

    ctx = ExitStack()
    with ctx:
        warm_sb = ctx.enter_context(nc.sbuf_tensor([128, 128], BF16))
        xs = ctx.enter_context(nc.sbuf_tensor([128, 2 + C, 512], BF16))
        vs_f = ctx.enter_context(nc.sbuf_tensor([128, VFR, 512], BF16))
        vs_sd = ctx.enter_context(nc.sbuf_tensor([128, SDR, 2, 2, 256], BF16))
        os_ = ctx.enter_context(nc.sbuf_tensor([128, P, 2, 512], BF16))
        vp = ctx.enter_context(nc.psum_tensor([128, VPR, 512], F32))
        op = ctx.enter_context(nc.psum_tensor([128, OPR, 512], F32))

        in_sems = [
            ctx.enter_context(nc.semaphore(f"in_sem{i}"))
            for i in range(len(IN_CHUNKS))
        ]
        pe_sem = ctx.enter_context(nc.semaphore("pe_sem"))
        dve_sem = ctx.enter_context(nc.semaphore("dve_sem"))
        act_sem = ctx.enter_context(nc.semaphore("act_sem"))
        out_sem = ctx.enter_context(nc.semaphore("out_sem"))
        warm_sem = ctx.enter_context(nc.semaphore("warm_sem"))
        sem_of = {"dve": dve_sem, "act": act_sem}
        count_of = {"dve": dve_count, "act": act_count}

        block = ctx.enter_context(nc.Block())

        @block.sync
        def _(eng):
            u0 = 0
            for ci, n in enumerate(IN_CHUNKS):
                eng.dma_start(
                    xs[:, u0 : u0 + n, :], x[:, u0 : u0 + n, :]
                ).then_inc(in_sems[ci], 16)
                u0 += n
            c0 = 0
            for n in OUT_CHUNKS:
                for eng_name in ("dve", "act"):
                    need = max(
                        (
                            count_of[eng_name][("out", q)]
                            for q in range(c0, c0 + n)
                            if OUT_ENG[q] == eng_name
                        ),
                        default=0,
                    )
                    if need:
                        eng.wait_ge(sem_of[eng_name], need)
                eng.dma_start(
                    out[:, c0 : c0 + n, :, :], os_[:, c0 : c0 + n, :, :]
                ).then_inc(out_sem, 16)
                c0 += n
            eng.wait_ge(out_sem, 16 * (len(OUT_CHUNKS) + 1))

        @block.tensor
        def _(eng):
            if sim:
                eng.wait_ge(warm_sem, 1)
            for _ in range(N_WARM):
                nc.tensor.matmul(
                    vp[:, 0, 0:128], warm_sb[:], warm_sb[:],
                    start=True, stop=True,
                )
            eng.wait_ge(in_sems[0], 16)
            seen_chunks = {0}
            for kind, i in pe:
                if kind == "S1":
                    s = i
                    ci = _chunk_of_slice(s)
                    if ci not in seen_chunks:
                        seen_chunks.add(ci)
                        eng.wait_ge(in_sems[ci], 16)
                    if s % 2 == 0 and s >= 4:
                        # vp banks for pair s//2 freed by the casts of
                        # slices s-4, s-3; this wait is the PSUM
                        # two-agent guard
                        eng.wait_ge(act_sem, act_count[("cast", s - 3)])
                    r = s % VPR
                    for mi in range(2):
                        for ki in range(2):
                            mm = nc.tensor.matmul(
                                vp[:, r, mi * 256 : (mi + 1) * 256],
                                xs[:, 2 + s, ki * 256 + mi * 128 : ki * 256 + (mi + 1) * 128],
                                xs[:, 0, ki * 256 : (ki + 1) * 256],
                                start=(ki == 0),
                                stop=(ki == 1),
                            )
                    mm.then_inc(pe_sem, 1)
                else:
                    q = i
                    # fold(q) done implies its casts and upstream;
                    # op-bank recycle needs out(q-2) explicitly
                    eng.wait_ge(dve_sem, dve_count[("fold", q)])
                    if q >= 2:
                        e = OUT_ENG[q - 2]
                        eng.wait_ge(sem_of[e], count_of[e][("out", q - 2)])
                    b0 = 2 * (q % 2)
                    nc.tensor.matmul(
                        op[:, b0, :],
                        xs[:, 1, 0:128],
                        vs_sd[:, q % SDR, 0, :, :],
                        start=True, stop=True,
                    )
                    mm = nc.tensor.matmul(
                        op[:, b0 + 1, :],
                        xs[:, 1, 128:256],
                        vs_sd[:, q % SDR, 1, :, :],
                        start=True, stop=True,
                    )
                    mm.then_inc(pe_sem, 1)

        @block.scalar
        def _(eng):
            for kind, i in act:
                if kind == "cast":
                    s = i
                    eng.wait_ge(pe_sem, pe_count[("S1", s)])
                    cp = nc.scalar.copy(
                        vs_f[:, s % VFR, :],
                        vp[:, s % VPR, :],
                    )
                else:
                    p = i
                    eng.wait_ge(pe_sem, pe_count[("S2", p)])
                    cp = nc.scalar.copy(
                        os_[:, p, :, :],
                        op[:, 2 * (p % 2) : 2 * (p % 2) + 2, :],
                    )
                cp.then_inc(act_sem, 1)
            # tail out-DMA for the last pair (its eviction just ran here)
            eng.dma_start(
                out[:, TAIL_PAIR, :, :], os_[:, TAIL_PAIR, :, :]
            ).then_inc(out_sem, 16)

        @block.vector
        def _(eng):
            add = mybir.AluOpType.add
            sub = mybir.AluOpType.subtract
            if sim:
                nc.vector.memset(warm_sb[:], 0.0).then_inc(warm_sem, 1)
            for kind, i in dve:
                if kind == "fold":
                    p = i
                    eng.wait_ge(act_sem, act_count[("cast", 2 * p + 1)])
                    f0 = (2 * p) % VFR
                    nc.vector.tensor_tensor(
                        vs_sd[:, p % SDR, 0, :, :],
                        vs_f[:, f0 : f0 + 2, 0:256],
                        vs_f[:, f0 : f0 + 2, 256:512],
                        add,
                    )
                    tt = nc.vector.tensor_tensor(
                        vs_sd[:, p % SDR, 1, :, :],
                        vs_f[:, f0 : f0 + 2, 0:256],
                        vs_f[:, f0 : f0 + 2, 256:512],
                        sub,
                    )
                    tt.then_inc(dve_sem, 1)
                else:
                    q = i
                    eng.wait_ge(pe_sem, pe_count[("S2", q)])
                    nc.vector.tensor_copy(
                        os_[:, q, :, :],
                        op[:, 2 * (q % 2) : 2 * (q % 2) + 2, :],
                    ).then_inc(dve_sem, 1)

    nc.compile()
    return nc


_NC_CACHE: bass.Bass | None = None


def _get_nc() -> bass.Bass:
    global _NC_CACHE
    if _NC_CACHE is None:
        _NC_CACHE = _build()
    return _NC_CACHE


def _make_in_maps(ip: np.ndarray) -> list[dict[str, np.ndarray]]:
    a = _dct_matrix()                                   # [256, 256] f32
    a_bf = a.astype(NP_BF16)
    unit_a = (
        a_bf.reshape(2, 128, 256).transpose(1, 0, 2).reshape(128, 512)
    )                                                   # [p, ki*256+j]
    unit_eo = np.zeros((128, 512), dtype=NP_BF16)
    unit_eo[:, 0:128] = a_bf[0:128, 0::2]               # E2[v, t']
    unit_eo[:, 128:256] = a_bf[0:128, 1::2]             # O2[v, t']
    in_maps = []
    for b in range(N_CORES):
        xb = ip[b].astype(NP_BF16)                      # [C, 256, 256]
        # w-permutation: cols 128.. hold w = 255..128
        xp = np.concatenate([xb[:, :, :128], xb[:, :, 128:][:, :, ::-1]], axis=2)
        # [s, ki, p, mi, c] -> [p, s, ki*256+mi*128+c]
        st = xp.reshape(C, 2, 128, 2, 128).transpose(2, 0, 1, 3, 4).reshape(128, C, 512)
        full = np.concatenate(
            [unit_a[:, None, :], unit_eo[:, None, :], st], axis=1
        )                                               # [128, 34, 512]
        in_maps.append({"x": np.ascontiguousarray(full)})
    return in_maps


def _unpack_out(results: list[dict[str, np.ndarray]]) -> np.ndarray:
    outs = []
    for b in range(N_CORES):
        o = np.asarray(results[b]["out"]).astype(np.float32)  # [128,16,2,512]
        o = o.reshape(128, P, 2, 2, 256)                # [t', pair, eo, sb, j]
        o = o.transpose(1, 3, 4, 0, 2).reshape(C, 256, 256)  # [s, j, w'=2t'+eo]
        outs.append(o)
    return np.stack(outs, axis=0)


def run(ip: np.ndarray, trace: bool = False):
    """Run the device kernel; returns (output, BassKernelResults)."""
    ip = np.asarray(ip)
    assert ip.shape == (N_CORES, C, 256, 256), ip.shape
    res = run_bass_kernel_spmd(
        _get_nc(), _make_in_maps(ip), core_ids=list(range(N_CORES)), trace=trace
    )
    return _unpack_out(res.results), res


def kernel(ip: np.ndarray) -> np.ndarray:
    out, _ = run(ip)
    return out


# revision 12
# speedup vs baseline: 1.0984x; 1.0218x over previous
"""2D DCT [8,32,256,256] on 8 TRN2 NeuronCores — raw Bass (no Tile).

Math: with A[m,k] = cos(pi*k*(m+0.5)/L)/L the 2D DCT per [256,256] slice is
    out = A^T @ X @ A
Stage 1: V = X^T A via 4 matmuls N=256 per slice (lhsT = X h-chunks,
rhs = A), one PSUM bank per slice. The host stages the second half of the
w columns REVERSED, so the bank holds
    vp[v, 0:256]   = v0 = V[v, j]        (v = 0..127)
    vp[v, 256:512] = v1 = V[255-v, j]
Stage 2 uses the DCT-II even/odd symmetry A[255-v, w'] = (-1)^w' A[v, w']:
    out[j, 2t']   = E2^T (v0 + v1),   E2[v,t'] = A[v, 2t']
    out[j, 2t'+1] = O2^T v0 - O2^T v1, O2[v,t'] = A[v, 2t'+1]
Per slice PAIR stage 2 is 3 matmuls of N=512 (contraction 128): the even
half consumes a DVE-folded s_w = v0+v1 (bf16, 2x-mode tensor_tensor); the
odd half does the subtract INSIDE PSUM accumulation using a staged -O2
(f32-exact, no fold needed). 1536+216 streamed PE columns per slice vs
2048 for the dense baseline, while the vector engines carry only
casts + one fold + out-evictions (~20us each, well under the PE's ~25us)
so the PE is self-paced — cross-engine hiccups don't propagate.

Pipeline per pair p (slices a=2p, b=2p+1):
    PE  S1(a), S1(b)          -> vp banks a%4, b%4  (4 MMs N=256 each)
    ACT cast(s) FD=512        vp bank -> vf[v0-group | v1-group] bf16
    DVE fold_s(p) FD=512 2x   vf v0,v1 -> vs_s (s_w pair, contiguous)
    PE  S2(p): E2^T s_w (N=512); O2^T v0pair - O2n^T v1pair (2 MMs N=512)
    DVE out-evict (ACT for pairs 3,9,15) op banks -> os bf16 FD=1024
    sync-ring DMA os -> DRAM (ACT DMAs the tail pair inline)

Wait discipline (waits break the LDWEIGHTS pull-ahead): PE block p =
[wait act>=cast(2p-3): vp two-agent guard, also implies S2(p-2)'s casts]
S1(2p) S1(2p+1) [wait dve>=out(p-4) (implies fold_s(p-2)) or fold_s(p-2);
ACT-assigned out(p-4) implied via the act wait] S2(p-2). ACT stream:
cast(s) ascending with out(q) right after cast(2q+3). DVE: fold_s(p)
then out(p-2). Never two agents on one PSUM bank concurrently.

Measured paces (this container, warm K=8/8 @2.4GHz): N=256 MM 109ns,
N=512 MM 216ns, ACT copy FD/1.2+143ns, DVE cast FD/0.96+65ns, DVE bf16
TT 2x FD/1.92+69ns. HAM: PE cold (1.2GHz) until ~3.4us of sustained
work — N_WARM=40 garbage matmuls bridge the DMA head so real S1s start
warm.
"""

import numpy as np

import concourse.bacc as bacc
import concourse.bass as bass
import concourse.mybir as mybir
from concourse.bass_utils import run_bass_kernel_spmd

N_CORES = 8
C = 32                    # slices per core
P = 16                    # slice pairs per core
L = 256
BF16 = mybir.dt.bfloat16
F32 = mybir.dt.float32
NP_BF16 = mybir.dt.np(mybir.dt.bfloat16)

# staged input units: 0 = A, 1 = [E2|O2|O2n|pad], 2+s = slice s
IN_CHUNKS = [3, 1, 1, 1, 2, 2, 3, 5, 8, 8]        # 34 units
OUT_CHUNKS = [3, 3, 3, 3, 2, 1]                   # pairs 0..14 on sync ring
TAIL_PAIR = 15                                    # pair 15 DMA'd from ACT
N_WARM = 40
VPR = 4                   # vp ring (banks) — slice s -> bank s%4
OPR = 4                   # op ring — pair p -> banks 2*(p%2), 2*(p%2)+1
VFR = 8                   # vf ring slots — slice s -> slot s%8
VSR = 6                   # vs_s ring — pair p -> slot p%6
LAG = 2                   # S2(p-LAG) in PE pair block p
OUT_ENG = ["act" if p % 6 == 3 or p == TAIL_PAIR else "dve" for p in range(P)]


def _dct_matrix() -> np.ndarray:
    m = np.arange(L, dtype=np.float64)
    k = np.arange(L, dtype=np.float64)
    a = np.cos(np.pi * np.outer(m + 0.5, k) / L) / L
    return a.astype(np.float32)


def _chunk_of_slice(s):
    u = s + 2
    c0 = 0
    for ci, n in enumerate(IN_CHUNKS):
        if u < c0 + n:
            return ci
        c0 += n
    raise AssertionError


def _schedules():
    """Per-engine op orders + completion counts (sem value when done)."""
    pe = []
    for p in range(P):
        pe.append(("S1", 2 * p))
        pe.append(("S1", 2 * p + 1))
        if p >= LAG:
            pe.append(("S2", p - LAG))
    for p in range(P - LAG, P):
        pe.append(("S2", p))
    pe_count = {o: i + 1 for i, o in enumerate(pe)}

    # ACT: casts ascending; out(q) placed right after cast(2q+3) so the
    # PE block's act>=cast(2p-3) wait transitively covers ACT outs
    act = []
    for s in range(2 * P):
        act.append(("cast", s))
        if s >= 3 and s % 2 == 1:
            q = (s - 3) // 2
            if OUT_ENG[q] == "act":
                act.append(("out", q))
    for q in (P - 2, P - 1):
        if OUT_ENG[q] == "act":
            act.append(("out", q))
    act_count = {o: i + 1 for i, o in enumerate(act)}

    # DVE: fold_s(p) leads, out(p-2) trails
    dve = []
    for p in range(P):
        dve.append(("fold", p))
        q = p - 2
        if q >= 0 and OUT_ENG[q] == "dve":
            dve.append(("out", q))
    for q in (P - 2, P - 1):
        if OUT_ENG[q] == "dve":
            dve.append(("out", q))
    dve_count = {o: i + 1 for i, o in enumerate(dve)}
    return pe, pe_count, act, act_count, dve, dve_count


def _build(sim: bool = False) -> bass.Bass:
    nc = bacc.Bacc()
    x = nc.declare_dram_parameter("x", [128, 2 + C, 512], BF16, isOutput=False)
    out = nc.declare_dram_parameter("out", [128, P, 2, 512], BF16, isOutput=True)

    pe, pe_count, act, act_count, dve, dve_count = _schedules()

    from contextlib import ExitStack

    ctx = ExitStack()
    with ctx:
        warm_sb = ctx.enter_context(nc.sbuf_tensor([128, 128], BF16))
        xs = ctx.enter_context(nc.sbuf_tensor([128, 2 + C, 512], BF16))
        # vf[:, 0, slot, :] = v0 of slice, vf[:, 1, slot, :] = v1
        vf = ctx.enter_context(nc.sbuf_tensor([128, 2, VFR, 256], BF16))
        vs_s = ctx.enter_context(nc.sbuf_tensor([128, VSR, 2, 256], BF16))
        os_ = ctx.enter_context(nc.sbuf_tensor([128, P, 2, 512], BF16))
        vp = ctx.enter_context(nc.psum_tensor([128, VPR, 512], F32))
        op = ctx.enter_context(nc.psum_tensor([128, OPR, 512], F32))

        in_sems = [
            ctx.enter_context(nc.semaphore(f"in_sem{i}"))
            for i in range(len(IN_CHUNKS))
        ]
        pe_sem = ctx.enter_context(nc.semaphore("pe_sem"))
        dve_sem = ctx.enter_context(nc.semaphore("dve_sem"))
        act_sem = ctx.enter_context(nc.semaphore("act_sem"))
        out_sem = ctx.enter_context(nc.semaphore("out_sem"))
        warm_sem = ctx.enter_context(nc.semaphore("warm_sem"))
        sem_of = {"dve": dve_sem, "act": act_sem}
        count_of = {"dve": dve_count, "act": act_count}

        block = ctx.enter_context(nc.Block())

        @block.sync
        def _(eng):
            u0 = 0
            for ci, n in enumerate(IN_CHUNKS):
                eng.dma_start(
                    xs[:, u0 : u0 + n, :], x[:, u0 : u0 + n, :]
                ).then_inc(in_sems[ci], 16)
                u0 += n
            c0 = 0
            for n in OUT_CHUNKS:
                for eng_name in ("dve", "act"):
                    need = max(
                        (
                            count_of[eng_name][("out", q)]
                            for q in range(c0, c0 + n)
                            if OUT_ENG[q] == eng_name
                        ),
                        default=0,
                    )
                    if need:
                        eng.wait_ge(sem_of[eng_name], need)
                eng.dma_start(
                    out[:, c0 : c0 + n, :, :], os_[:, c0 : c0 + n, :, :]
                ).then_inc(out_sem, 16)
                c0 += n
            eng.wait_ge(out_sem, 16 * (len(OUT_CHUNKS) + 1))

        @block.tensor
        def _(eng):
            if sim:
                eng.wait_ge(warm_sem, 1)
            for _ in range(N_WARM):
                nc.tensor.matmul(
                    vp[:, 0, 0:128], warm_sb[:], warm_sb[:],
                    start=True, stop=True,
                )
            eng.wait_ge(in_sems[0], 16)
            seen_chunks = {0}
            for kind, i in pe:
                if kind == "S1":
                    s = i
                    ci = _chunk_of_slice(s)
                    if ci not in seen_chunks:
                        seen_chunks.add(ci)
                        eng.wait_ge(in_sems[ci], 16)
                    if s % 2 == 0 and s >= 4:
                        # vp two-agent guard: bank freed by cast(s-3);
                        # also implies everything S2(s//2 - 2) needs
                        # from the ACT stream
                        eng.wait_ge(act_sem, act_count[("cast", s - 3)])
                    r = s % VPR
                    for mi in range(2):
                        for ki in range(2):
                            mm = nc.tensor.matmul(
                                vp[:, r, mi * 256 : (mi + 1) * 256],
                                xs[:, 2 + s, ki * 256 + mi * 128 : ki * 256 + (mi + 1) * 128],
                                xs[:, 0, ki * 256 : (ki + 1) * 256],
                                start=(ki == 0),
                                stop=(ki == 1),
                            )
                    mm.then_inc(pe_sem, 1)
                else:
                    q = i
                    if q >= 2 and OUT_ENG[q - 2] == "dve":
                        # implies fold_s(q) done too (stream order)
                        eng.wait_ge(dve_sem, dve_count[("out", q - 2)])
                    else:
                        eng.wait_ge(dve_sem, dve_count[("fold", q)])
                    f0 = (2 * q) % VFR
                    b0 = 2 * (q % 2)
                    nc.tensor.matmul(
                        op[:, b0, :],
                        xs[:, 1, 0:128],
                        vs_s[:, q % VSR, :, :],
                        start=True, stop=True,
                    )
                    nc.tensor.matmul(
                        op[:, b0 + 1, :],
                        xs[:, 1, 128:256],
                        vf[:, 0, f0 : f0 + 2, :],
                        start=True, stop=False,
                    )
                    mm = nc.tensor.matmul(
                        op[:, b0 + 1, :],
                        xs[:, 1, 256:384],
                        vf[:, 1, f0 : f0 + 2, :],
                        start=False, stop=True,
                    )
                    mm.then_inc(pe_sem, 1)

        @block.scalar
        def _(eng):
            for kind, i in act:
                if kind == "cast":
                    s = i
                    eng.wait_ge(pe_sem, pe_count[("S1", s)])
                    cp = nc.scalar.copy(
                        vf[:, :, s % VFR, :],
                        vp[:, s % VPR, :],
                    )
                else:
                    q = i
                    eng.wait_ge(pe_sem, pe_count[("S2", q)])
                    cp = nc.scalar.copy(
                        os_[:, q, :, :],
                        op[:, 2 * (q % 2) : 2 * (q % 2) + 2, :],
                    )
                cp.then_inc(act_sem, 1)
            # tail out-DMA for the last pair (evicted just above on ACT)
            eng.dma_start(
                out[:, TAIL_PAIR, :, :], os_[:, TAIL_PAIR, :, :]
            ).then_inc(out_sem, 16)

        @block.vector
        def _(eng):
            add = mybir.AluOpType.add
            if sim:
                nc.vector.memset(warm_sb[:], 0.0).then_inc(warm_sem, 1)
            for kind, i in dve:
                if kind == "fold":
                    p = i
                    eng.wait_ge(act_sem, act_count[("cast", 2 * p + 1)])
                    f0 = (2 * p) % VFR
                    nc.vector.tensor_tensor(
                        vs_s[:, p % VSR, :, :],
                        vf[:, 0, f0 : f0 + 2, :],
                        vf[:, 1, f0 : f0 + 2, :],
                        add,
                    ).then_inc(dve_sem, 1)
                else:
                    q = i
                    eng.wait_ge(pe_sem, pe_count[("S2", q)])
                    nc.vector.tensor_copy(
                        os_[:, q, :, :],
                        op[:, 2 * (q % 2) : 2 * (q % 2) + 2, :],
                    ).then_inc(dve_sem, 1)

    nc.compile()
    return nc


_NC_CACHE: bass.Bass | None = None


def _get_nc() -> bass.Bass:
    global _NC_CACHE
    if _NC_CACHE is None:
        _NC_CACHE = _build()
    return _NC_CACHE


def _make_in_maps(ip: np.ndarray) -> list[dict[str, np.ndarray]]:
    a = _dct_matrix()                                   # [256, 256] f32
    a_bf = a.astype(NP_BF16)
    unit_a = (
        a_bf.reshape(2, 128, 256).transpose(1, 0, 2).reshape(128, 512)
    )                                                   # [p, ki*256+j]
    unit_eo = np.zeros((128, 512), dtype=NP_BF16)
    unit_eo[:, 0:128] = a_bf[0:128, 0::2]               # E2[v, t']
    unit_eo[:, 128:256] = a_bf[0:128, 1::2]             # O2[v, t']
    unit_eo[:, 256:384] = -a_bf[0:128, 1::2]            # -O2
    in_maps = []
    for b in range(N_CORES):
        xb = ip[b].astype(NP_BF16)                      # [C, 256, 256]
        # w-permutation: cols 128.. hold w = 255..128
        xp = np.concatenate([xb[:, :, :128], xb[:, :, 128:][:, :, ::-1]], axis=2)
        # [s, ki, p, mi, c] -> [p, s, ki*256+mi*128+c]
        st = xp.reshape(C, 2, 128, 2, 128).transpose(2, 0, 1, 3, 4).reshape(128, C, 512)
        full = np.concatenate(
            [unit_a[:, None, :], unit_eo[:, None, :], st], axis=1
        )                                               # [128, 34, 512]
        in_maps.append({"x": np.ascontiguousarray(full)})
    return in_maps


def _unpack_out(results: list[dict[str, np.ndarray]]) -> np.ndarray:
    outs = []
    for b in range(N_CORES):
        o = np.asarray(results[b]["out"]).astype(np.float32)  # [128,16,2,512]
        o = o.reshape(128, P, 2, 2, 256)                # [t', pair, eo, sb, j]
        o = o.transpose(1, 3, 4, 0, 2).reshape(C, 256, 256)  # [s, j, w'=2t'+eo]
        outs.append(o)
    return np.stack(outs, axis=0)


def run(ip: np.ndarray, trace: bool = False):
    """Run the device kernel; returns (output, BassKernelResults)."""
    ip = np.asarray(ip)
    assert ip.shape == (N_CORES, C, 256, 256), ip.shape
    res = run_bass_kernel_spmd(
        _get_nc(), _make_in_maps(ip), core_ids=list(range(N_CORES)), trace=trace
    )
    return _unpack_out(res.results), res


def kernel(ip: np.ndarray) -> np.ndarray:
    out, _ = run(ip)
    return out


# revision 17
# speedup vs baseline: 1.1806x; 1.0748x over previous
"""2D DCT [8,32,256,256] on 8 TRN2 NeuronCores — raw Bass (no Tile).

Math: with A[m,k] = cos(pi*k*(m+0.5)/L)/L the 2D DCT per [256,256] slice is
    out = A^T @ X @ A
Stage 1: V = X^T A via 4 matmuls N=256 per slice (lhsT = X h-chunks,
rhs = A), one PSUM bank per slice. The host stages the second half of the
w columns REVERSED, so the bank holds
    vp[v, 0:256]   = v0 = V[v, j]        (v = 0..127)
    vp[v, 256:512] = v1 = V[255-v, j]
Stage 2 uses the DCT-II even/odd symmetry A[255-v, w'] = (-1)^w' A[v, w']:
    out[j, 2t']   = E2^T (v0 + v1),   E2[v,t'] = A[v, 2t']
    out[j, 2t'+1] = O2^T v0 - O2^T v1, O2[v,t'] = A[v, 2t'+1]
Per slice PAIR stage 2 is 3 matmuls of N=512 (contraction 128): the even
half consumes a DVE-folded s_w = v0+v1 (bf16, 2x-mode tensor_tensor); the
odd half does the subtract INSIDE PSUM accumulation using a staged -O2
(f32-exact, no fold needed). 1536+216 streamed PE columns per slice vs
2048 for the dense baseline, while the vector engines carry only
casts + one fold + out-evictions (~20us each, well under the PE's ~25us)
so the PE is self-paced — cross-engine hiccups don't propagate.

Pipeline per pair p (slices a=2p, b=2p+1):
    PE  S1(a), S1(b)          -> vp banks a%4, b%4  (4 MMs N=256 each)
    ACT cast(s) FD=512        vp bank -> vf[v0-group | v1-group] bf16
    DVE fold_s(p) FD=512 2x   vf v0,v1 -> vs_s (s_w pair, contiguous)
    PE  S2(p): E2^T s_w (N=512); O2^T v0pair - O2n^T v1pair (2 MMs N=512)
    DVE out-evict (ACT for pairs 3,9,15) op banks -> os bf16 FD=1024
    sync-ring DMA os -> DRAM (ACT DMAs the tail pair inline)

Wait discipline (waits break the LDWEIGHTS pull-ahead): PE block p =
[wait act>=cast(2p-3): vp two-agent guard, also implies S2(p-2)'s casts]
S1(2p) S1(2p+1) [wait dve>=out(p-4) (implies fold_s(p-2)) or fold_s(p-2);
ACT-assigned out(p-4) implied via the act wait] S2(p-2). ACT stream:
cast(s) ascending with out(q) right after cast(2q+3). DVE: fold_s(p)
then out(p-2). Never two agents on one PSUM bank concurrently.

Measured paces (this container, warm K=8/8 @2.4GHz): N=256 MM 109ns,
N=512 MM 216ns, ACT copy FD/1.2+143ns, DVE cast FD/0.96+65ns, DVE bf16
TT 2x FD/1.92+69ns. HAM: PE cold (1.2GHz) until ~3.4us of sustained
work — N_WARM=40 garbage matmuls bridge the DMA head so real S1s start
warm.
"""

import numpy as np

import concourse.bacc as bacc
import concourse.bass as bass
import concourse.mybir as mybir
from concourse.bass_utils import run_bass_kernel_spmd

N_CORES = 8
C = 32                    # slices per core
P = 16                    # slice pairs per core
L = 256
BF16 = mybir.dt.bfloat16
F32 = mybir.dt.float32
NP_BF16 = mybir.dt.np(mybir.dt.bfloat16)

# staged input units: 0 = A, 1 = [E2|O2|O2n|pad], 2+s = slice s
IN_CHUNKS = [2, 1, 1, 1, 1, 1, 1, 2, 2, 2, 3, 4, 5, 8]   # 34 units
OUT_CHUNKS = [3, 3, 3, 3, 2]                      # pairs 0..13 on sync ring
TAIL_PAIR = 15                                    # pairs 14+15 DMA'd from ACT
N_WARM = 34
VPR = 4                   # vp ring (banks) — slice s -> bank s%4
OPR = 4                   # op ring — pair p -> banks 2*(p%2), 2*(p%2)+1
VFR = 8                   # vf ring slots — slice s -> slot s%8
VSR = 6                   # vs_s ring — pair p -> slot p%6
LAG = 2                   # S2(p-LAG) in PE pair block p
OUT_ENG = ["act" if p == TAIL_PAIR else "dve" for p in range(P)]


def _dct_matrix() -> np.ndarray:
    m = np.arange(L, dtype=np.float64)
    k = np.arange(L, dtype=np.float64)
    a = np.cos(np.pi * np.outer(m + 0.5, k) / L) / L
    return a.astype(np.float32)


def _chunk_of_slice(s):
    u = s + 2
    c0 = 0
    for ci, n in enumerate(IN_CHUNKS):
        if u < c0 + n:
            return ci
        c0 += n
    raise AssertionError


def _schedules():
    """Per-engine op orders + completion counts (sem value when done)."""
    pe = []
    for p in range(P):
        pe.append(("S1", 2 * p))
        pe.append(("S1", 2 * p + 1))
        if p >= LAG:
            pe.append(("S2", p - LAG))
    for p in range(P - LAG, P):
        pe.append(("S2", p))
    pe_count = {o: i + 1 for i, o in enumerate(pe)}

    # ACT: casts ascending; out(q) placed right after cast(2q+3) so the
    # PE block's act>=cast(2p-3) wait transitively covers ACT outs
    act = []
    for s in range(2 * P):
        act.append(("cast", s))
        if s >= 3 and s % 2 == 1:
            q = (s - 3) // 2
            if OUT_ENG[q] == "act":
                act.append(("out", q))
    for q in (P - 2, P - 1):
        if OUT_ENG[q] == "act":
            act.append(("out", q))
    act_count = {o: i + 1 for i, o in enumerate(act)}

    # DVE: fold_s(p) leads, out(p-2) trails
    dve = []
    for p in range(P):
        dve.append(("fold", p))
        q = p - 2
        if q >= 0 and OUT_ENG[q] == "dve":
            dve.append(("out", q))
    for q in (P - 2, P - 1):
        if OUT_ENG[q] == "dve":
            dve.append(("out", q))
    dve_count = {o: i + 1 for i, o in enumerate(dve)}
    return pe, pe_count, act, act_count, dve, dve_count


def _build(sim: bool = False) -> bass.Bass:
    nc = bacc.Bacc()
    x = nc.declare_dram_parameter("x", [128, 2 + C, 512], BF16, isOutput=False)
    out = nc.declare_dram_parameter("out", [128, P, 2, 512], BF16, isOutput=True)

    pe, pe_count, act, act_count, dve, dve_count = _schedules()

    from contextlib import ExitStack

    ctx = ExitStack()
    with ctx:
        warm_sb = ctx.enter_context(nc.sbuf_tensor([128, 128], BF16))
        xs = ctx.enter_context(nc.sbuf_tensor([128, 2 + C, 512], BF16))
        # vf[:, 0, slot, :] = v0 of slice, vf[:, 1, slot, :] = v1
        vf = ctx.enter_context(nc.sbuf_tensor([128, 2, VFR, 256], BF16))
        vs_s = ctx.enter_context(nc.sbuf_tensor([128, VSR, 2, 256], BF16))
        os_ = ctx.enter_context(nc.sbuf_tensor([128, P, 2, 512], BF16))
        vp = ctx.enter_context(nc.psum_tensor([128, VPR, 512], F32))
        op = ctx.enter_context(nc.psum_tensor([128, OPR, 512], F32))

        in_sems = [
            ctx.enter_context(nc.semaphore(f"in_sem{i}"))
            for i in range(len(IN_CHUNKS))
        ]
        pe_sem = ctx.enter_context(nc.semaphore("pe_sem"))
        dve_sem = ctx.enter_context(nc.semaphore("dve_sem"))
        act_sem = ctx.enter_context(nc.semaphore("act_sem"))
        out_sem = ctx.enter_context(nc.semaphore("out_sem"))
        warm_sem = ctx.enter_context(nc.semaphore("warm_sem"))
        sem_of = {"dve": dve_sem, "act": act_sem}
        count_of = {"dve": dve_count, "act": act_count}

        block = ctx.enter_context(nc.Block())

        @block.sync
        def _(eng):
            u0 = 0
            for ci, n in enumerate(IN_CHUNKS):
                eng.dma_start(
                    xs[:, u0 : u0 + n, :], x[:, u0 : u0 + n, :]
                ).then_inc(in_sems[ci], 16)
                u0 += n
            c0 = 0
            for n in OUT_CHUNKS:
                for eng_name in ("dve", "act"):
                    need = max(
                        (
                            count_of[eng_name][("out", q)]
                            for q in range(c0, c0 + n)
                            if OUT_ENG[q] == eng_name
                        ),
                        default=0,
                    )
                    if need:
                        eng.wait_ge(sem_of[eng_name], need)
                eng.dma_start(
                    out[:, c0 : c0 + n, :, :], os_[:, c0 : c0 + n, :, :]
                ).then_inc(out_sem, 16)
                c0 += n
            eng.wait_ge(out_sem, 16 * (len(OUT_CHUNKS) + 1))

        @block.tensor
        def _(eng):
            if sim:
                eng.wait_ge(warm_sem, 1)
            for _ in range(N_WARM):
                nc.tensor.matmul(
                    vp[:, 0, 0:128], warm_sb[:], warm_sb[:],
                    start=True, stop=True,
                )
            eng.wait_ge(in_sems[0], 16)
            seen_chunks = {0}
            for kind, i in pe:
                if kind == "S1":
                    s = i
                    ci = _chunk_of_slice(s)
                    if ci not in seen_chunks:
                        seen_chunks.add(ci)
                        eng.wait_ge(in_sems[ci], 16)
                    if s % 2 == 0 and s >= 4:
                        # vp two-agent guard: bank freed by cast(s-3);
                        # also implies everything S2(s//2 - 2) needs
                        # from the ACT stream
                        eng.wait_ge(act_sem, act_count[("cast", s - 3)])
                    r = s % VPR
                    for mi in range(2):
                        for ki in range(2):
                            mm = nc.tensor.matmul(
                                vp[:, r, mi * 256 : (mi + 1) * 256],
                                xs[:, 2 + s, ki * 256 + mi * 128 : ki * 256 + (mi + 1) * 128],
                                xs[:, 0, ki * 256 : (ki + 1) * 256],
                                start=(ki == 0),
                                stop=(ki == 1),
                            )
                    mm.then_inc(pe_sem, 1)
                else:
                    q = i
                    if q >= 2 and OUT_ENG[q - 2] == "dve":
                        # implies fold_s(q) done too (stream order)
                        eng.wait_ge(dve_sem, dve_count[("out", q - 2)])
                    else:
                        eng.wait_ge(dve_sem, dve_count[("fold", q)])
                    f0 = (2 * q) % VFR
                    b0 = 2 * (q % 2)
                    nc.tensor.matmul(
                        op[:, b0, :],
                        xs[:, 1, 0:128],
                        vs_s[:, q % VSR, :, :],
                        start=True, stop=True,
                    )
                    nc.tensor.matmul(
                        op[:, b0 + 1, :],
                        xs[:, 1, 128:256],
                        vf[:, 0, f0 : f0 + 2, :],
                        start=True, stop=False,
                    )
                    mm = nc.tensor.matmul(
                        op[:, b0 + 1, :],
                        xs[:, 1, 256:384],
                        vf[:, 1, f0 : f0 + 2, :],
                        start=False, stop=True,
                    )
                    mm.then_inc(pe_sem, 1)

        @block.scalar
        def _(eng):
            for kind, i in act:
                if kind == "cast":
                    s = i
                    eng.wait_ge(pe_sem, pe_count[("S1", s)])
                    cp = nc.scalar.copy(
                        vf[:, :, s % VFR, :],
                        vp[:, s % VPR, :],
                    )
                else:
                    q = i
                    eng.wait_ge(pe_sem, pe_count[("S2", q)])
                    cp = nc.scalar.copy(
                        os_[:, q, :, :],
                        op[:, 2 * (q % 2) : 2 * (q % 2) + 2, :],
                    )
                cp.then_inc(act_sem, 1)
            # tail out-DMA covers pairs 14 (evicted by DVE) and 15 (above)
            eng.wait_ge(dve_sem, dve_count[("out", P - 2)])
            eng.dma_start(
                out[:, P - 2 :, :, :], os_[:, P - 2 :, :, :]
            ).then_inc(out_sem, 16)

        @block.vector
        def _(eng):
            add = mybir.AluOpType.add
            if sim:
                nc.vector.memset(warm_sb[:], 0.0).then_inc(warm_sem, 1)
            for kind, i in dve:
                if kind == "fold":
                    p = i
                    eng.wait_ge(act_sem, act_count[("cast", 2 * p + 1)])
                    f0 = (2 * p) % VFR
                    nc.vector.tensor_tensor(
                        vs_s[:, p % VSR, :, :],
                        vf[:, 0, f0 : f0 + 2, :],
                        vf[:, 1, f0 : f0 + 2, :],
                        add,
                    ).then_inc(dve_sem, 1)
                else:
                    q = i
                    eng.wait_ge(pe_sem, pe_count[("S2", q)])
                    nc.vector.tensor_copy(
                        os_[:, q, :, :],
                        op[:, 2 * (q % 2) : 2 * (q % 2) + 2, :],
                    ).then_inc(dve_sem, 1)

    nc.compile()
    return nc


_NC_CACHE: bass.Bass | None = None


def _get_nc() -> bass.Bass:
    global _NC_CACHE
    if _NC_CACHE is None:
        _NC_CACHE = _build()
    return _NC_CACHE


def _make_in_maps(ip: np.ndarray) -> list[dict[str, np.ndarray]]:
    a = _dct_matrix()                                   # [256, 256] f32
    a_bf = a.astype(NP_BF16)
    unit_a = (
        a_bf.reshape(2, 128, 256).transpose(1, 0, 2).reshape(128, 512)
    )                                                   # [p, ki*256+j]
    unit_eo = np.zeros((128, 512), dtype=NP_BF16)
    unit_eo[:, 0:128] = a_bf[0:128, 0::2]               # E2[v, t']
    unit_eo[:, 128:256] = a_bf[0:128, 1::2]             # O2[v, t']
    unit_eo[:, 256:384] = -a_bf[0:128, 1::2]            # -O2
    in_maps = []
    for b in range(N_CORES):
        xb = ip[b].astype(NP_BF16)                      # [C, 256, 256]
        # w-permutation: cols 128.. hold w = 255..128
        xp = np.concatenate([xb[:, :, :128], xb[:, :, 128:][:, :, ::-1]], axis=2)
        # [s, ki, p, mi, c] -> [p, s, ki*256+mi*128+c]
        st = xp.reshape(C, 2, 128, 2, 128).transpose(2, 0, 1, 3, 4).reshape(128, C, 512)
        full = np.concatenate(
            [unit_a[:, None, :], unit_eo[:, None, :], st], axis=1
        )                                               # [128, 34, 512]
        in_maps.append({"x": np.ascontiguousarray(full)})
    return in_maps


def _unpack_out(results: list[dict[str, np.ndarray]]) -> np.ndarray:
    outs = []
    for b in range(N_CORES):
        o = np.asarray(results[b]["out"]).astype(np.float32)  # [128,16,2,512]
        o = o.reshape(128, P, 2, 2, 256)                # [t', pair, eo, sb, j]
        o = o.transpose(1, 3, 4, 0, 2).reshape(C, 256, 256)  # [s, j, w'=2t'+eo]
        outs.append(o)
    return np.stack(outs, axis=0)


def run(ip: np.ndarray, trace: bool = False):
    """Run the device kernel; returns (output, BassKernelResults)."""
    ip = np.asarray(ip)
    assert ip.shape == (N_CORES, C, 256, 256), ip.shape
    res = run_bass_kernel_spmd(
        _get_nc(), _make_in_maps(ip), core_ids=list(range(N_CORES)), trace=trace
    )
    return _unpack_out(res.results), res


def kernel(ip: np.ndarray) -> np.ndarray:
    out, _ = run(ip)
    return out
